# revision 1
# baseline (speedup 1.0000x reference)
"""Trainium2 Bass kernel for nn_DSVDD (retrieval_knn).

Math (per batch b):
  phi = W @ p_b + bias            [DIM, HW]    (1x1 conv)
  sqdist[i,j] = ||phi_i||^2 + ||C_j||^2 - 2 phi_i . C_j
  top-3 smallest distances d0<=d1<=d2  ->  w0 = 1/(1+exp(d0-d1)+exp(d0-d2))
  score[i] = w0 * d0

Device strategy (8 cores, data-parallel over (batch, HW-half)):
  Y[i,j] = 2 phi_i . C_j - ||C_j||^2.  The 2C part runs as fp32r PE matmuls;
  the -c_j correction is materialized once per j-slice ([128, js] via a
  ones-matmul on the replicated -c/128 block) and applied by a DVE add.
  top-3 smallest sqdist == top-3 largest Y (f_i = ||phi_i||^2 common per row).
  DVE max8 finds the top-8 largest Y per row in one instruction; streamed
  merge over j-slices.  f_i via ones-matmuls over Squared phi (deferred one
  conv step so they never stall the PE).  Tail (sqrt, softmin) on ACT/DVE.
"""
import sys

sys.path.insert(0, "/opt/trn_rl_repo")

import numpy as np

B, DIM, H, W_ = 4, 1792, 56, 56
HW = H * W_            # 3136
P = 3136               # prototypes
NCORES = 8
HALF = HW // 2         # 1568 positions per core
KC = DIM // 128        # 14 contraction chunks
KCH = KC // 2          # 7 (p tiles split in halves for early start)
KCC = KC + 1           # 15 chunks in cb (incl. replicated -c/128 block)
IB = 392               # conv i-block (moving cols)
NIB = HALF // IB       # 4
JSLICES = [256, 480, 480, 480, 480, 480, 480]   # G-phase j-slices (sum 3136)
NIT = 13               # i-tiles: 12 full + 1 ragged(32)
LAST_W = HALF - 12 * 128   # 32

_cache = {}


def _build_program():
    import concourse.tile as tile
    from concourse import bacc, mybir

    F32 = mybir.dt.float32
    F32R = mybir.dt.float32r
    AF = mybir.ActivationFunctionType
    ALU = mybir.AluOpType
    AX = mybir.AxisListType

    nc = bacc.Bacc("TRN2", target_bir_lowering=False, debug=False)

    pT_d = nc.dram_tensor("pT", [DIM, HALF], F32R, kind="ExternalInput")
    wt_d = nc.dram_tensor("wt", [DIM, DIM], F32R, kind="ExternalInput")   # W^T
    cb_d = nc.dram_tensor("cb", [KCC * 128, P], F32R, kind="ExternalInput")
    bias_d = nc.dram_tensor("bias", [DIM], F32, kind="ExternalInput")
    onec_d = nc.dram_tensor("onec", [128, 1], F32R, kind="ExternalInput")
    oner_d = nc.dram_tensor("oner", [1, 128], F32R, kind="ExternalInput")
    ones2_d = nc.dram_tensor("ones2", [128, 128], F32R, kind="ExternalInput")
    score_d = nc.dram_tensor("score", [128, NIT], F32, kind="ExternalOutput")

    with tile.TileContext(nc) as tc:
        with (
            tc.tile_pool(name="persist", bufs=1) as persist,
            tc.tile_pool(name="cbp0", bufs=1) as cbp0,
        ):
            phi = persist.tile([128, KC, HALF], F32R)
            bias_col = persist.tile([128, KC], F32)
            onec = persist.tile([128, 1], F32R)
            oner = persist.tile([1, 128], F32R)
            ones2 = persist.tile([128, 128], F32R)
            f_row = persist.tile([1, HALF], F32)
            f_col = persist.tile([128, NIT], F32)
            runA = persist.tile([128, NIT, 8], F32)
            score_col = persist.tile([128, NIT], F32)

            # ------------- conv phase: phi = W @ p + b, f = ||phi||^2 -------
            with (
                tc.tile_pool(name="pp", bufs=6) as pp,
                tc.tile_pool(name="wtp", bufs=3) as wtp,
                tc.tile_pool(name="sqp", bufs=4) as sqp,
                tc.tile_pool(name="cps", bufs=4, space="PSUM") as cps,
                tc.tile_pool(name="fps", bufs=1, space="PSUM") as fps,
            ):
                f_ps = [fps.tile([1, IB], F32, name=f"fp{ib}", tag=f"f{ib}")
                        for ib in range(NIB)]

                def load_wt(dcg):
                    t = wtp.tile([128, KC, 128], F32R, name="wt_t")
                    nc.sync.dma_start(
                        t[:],
                        wt_d[:, dcg * 128:(dcg + 1) * 128].rearrange(
                            "(cc p) d -> p cc d", p=128),
                    )
                    return t

                def load_phalf(ib, h):
                    t = pp.tile([128, KCH, IB], F32R, name=f"pq{ib}{h}",
                                tag="pq")
                    nc.sync.dma_start(
                        t[:],
                        pT_d[h * KCH * 128:(h + 1) * KCH * 128,
                             ib * IB:(ib + 1) * IB].rearrange(
                            "(cc p) i -> p cc i", p=128),
                    )
                    return t

                # startup-critical loads first: wt chunk 0, then p halves
                dcg_seq = list(range(KC)) + list(reversed(range(KC)))  # snake
                wt_tiles = {0: load_wt(dcg_seq[0])}
                wt_issued = 1

                def wt_prefetch(upto):
                    nonlocal wt_issued
                    while wt_issued < min(upto, 2 * KC):
                        if dcg_seq[wt_issued] == dcg_seq[wt_issued - 1]:
                            # snake turn: same chunk again, reuse the tile
                            wt_tiles[wt_issued] = wt_tiles[wt_issued - 1]
                        else:
                            wt_tiles[wt_issued] = load_wt(dcg_seq[wt_issued])
                        wt_issued += 1

                # PE warmup: dummy matmuls keep HAM's activity monitor hot
                # while the first real DMAs land, so conv starts at 2.4 GHz.
                warm = pp.tile([128, 512], F32R, name="warm", tag="warm", bufs=1)
                nc.vector.memset(warm[:].bitcast(F32), 1.0)
                wps = cps.tile([128, 512], F32, name="wps", tag="acc")
                for _ in range(68):
                    nc.tensor.matmul(wps[:], warm[:, 0:128], warm[:],
                                     start=True, stop=True)

                cb0_t = None
                small_dmas_done = False
                pending_f = []
                for sub in range(2):
                    p_t = {}
                    for ib in (2 * sub, 2 * sub + 1):
                        p_t[ib] = [load_phalf(ib, 0), load_phalf(ib, 1)]
                    if not small_dmas_done:
                        small_dmas_done = True
                        nc.sync.dma_start(
                            bias_col[:],
                            bias_d.rearrange("(g p) -> p g", p=128))
                        nc.sync.dma_start(onec[:], onec_d[:])
                        nc.sync.dma_start(oner[:], oner_d[:])
                        nc.sync.dma_start(ones2[:], ones2_d[:])
                    for dcg_i in range(KC):
                        pos = sub * KC + dcg_i
                        dcg = dcg_seq[pos]
                        wt_t = wt_tiles.pop(pos)
                        wt_prefetch(pos + 3)
                        for k, ib in enumerate((2 * sub, 2 * sub + 1)):
                            if k == 1 and pending_f:
                                # deferred f matmuls: deps long satisfied
                                for args, kw in pending_f:
                                    nc.tensor.matmul(*args, **kw)
                                pending_f = []
                            acc = cps.tile([128, IB], F32)
                            for cc in range(KC):
                                nc.tensor.matmul(
                                    acc[:],
                                    wt_t[:, cc, :],
                                    p_t[ib][cc // KCH][:, cc % KCH, :],
                                    start=(cc == 0),
                                    stop=(cc == KC - 1),
                                )
                            isl = slice(ib * IB, (ib + 1) * IB)
                            # phi = psum + bias (rounded to fp32r)
                            nc.scalar.activation(
                                phi[:, dcg, isl], acc[:], AF.Identity,
                                bias=bias_col[:, dcg:dcg + 1],
                            )
                            # phi2 = (psum + bias)^2
                            sq = sqp.tile([128, IB], F32R)
                            nc.scalar.activation(
                                sq[:], acc[:], AF.Square,
                                bias=bias_col[:, dcg:dcg + 1],
                            )
                            pending_f.append((
                                (f_ps[ib][:], onec[:], sq[:]),
                                dict(start=(dcg_i == 0), stop=(dcg_i == KC - 1)),
                            ))
                    if sub == 0:
                        # prefetch first G slice mid-conv
                        j0 = JSLICES[0]
                        cb0_t = cbp0.tile([128, KCC, j0], F32R)
                        nc.sync.dma_start(
                            cb0_t[:],
                            cb_d[:, 0:j0].rearrange("(cc p) j -> p cc j",
                                                    p=128),
                        )
                for args, kw in pending_f:
                    nc.tensor.matmul(*args, **kw)
                pending_f = []
                for ib in range(NIB):
                    nc.vector.tensor_copy(
                        f_row[:, ib * IB:(ib + 1) * IB], f_ps[ib][:]
                    )

            # ------------- f relayout: [1, 1568] -> [128, 13] ---------------
            with tc.tile_pool(name="ftp", bufs=2, space="PSUM") as ftp:
                ft = ftp.tile([128, NIT], F32)
                for it in range(NIT):
                    w = 128 if it < 12 else LAST_W
                    nc.tensor.transpose(
                        ft[0:w, it:it + 1],
                        f_row[:, it * 128:it * 128 + w],
                        oner[0:1, 0:1].bitcast(F32),
                    )
                nc.scalar.activation(f_col[:], ft[:], AF.Copy)

            # ------------- G phase: Y = 2 phi.C - c, streamed top-8 ---------
            with (
                tc.tile_pool(name="cbp", bufs=2) as cbp,
                tc.tile_pool(name="cbcp", bufs=2) as cbcp,
                tc.tile_pool(name="ysb", bufs=4) as ysb,
                tc.tile_pool(name="mrg", bufs=4) as mrg,
                tc.tile_pool(name="yps", bufs=8, space="PSUM") as yps,
            ):
                joff = [0]
                for js in range(1, len(JSLICES)):
                    joff.append(joff[-1] + JSLICES[js - 1])

                for js in range(len(JSLICES)):
                    w_js = JSLICES[js]
                    jsl = slice(joff[js], joff[js] + w_js)
                    if js == 0:
                        cb_t = cb0_t
                    else:
                        cb_t = cbp.tile([128, KCC, w_js], F32R, name="cb_t",
                                        tag="cb")
                        nc.sync.dma_start(
                            cb_t[:],
                            cb_d[:, jsl].rearrange("(cc p) j -> p cc j",
                                                   p=128),
                        )
                    # materialize -c for this slice: ones2 @ (-c/128 block)
                    cps_t = yps.tile([128, 512], F32, name="y", tag="y")
                    nc.tensor.matmul(cps_t[:, 0:w_js], ones2[:],
                                     cb_t[:, KC, :], start=True, stop=True)
                    cbc_t = cbcp.tile([128, 512], F32, name="cbc_t")
                    nc.scalar.activation(cbc_t[:, 0:w_js], cps_t[:, 0:w_js],
                                         AF.Copy)
                    for it in range(NIT):
                        w = 128 if it < 12 else LAST_W
                        i0 = it * 128
                        y = yps.tile([128, 512], F32, name="y", tag="y")
                        for cc in range(KC):
                            nc.tensor.matmul(
                                y[0:w, 0:w_js],
                                phi[:, cc, i0:i0 + w],
                                cb_t[:, cc, :],
                                start=(cc == 0),
                                stop=(cc == KC - 1),
                            )
                        ys = ysb.tile([128, 512], F32, name="ys", tag="ys")
                        nc.vector.tensor_tensor(
                            ys[0:w, 0:w_js], y[0:w, 0:w_js],
                            cbc_t[0:w, 0:w_js], ALU.add,
                        )
                        if js == 0:
                            nc.vector.max(runA[0:w, it, :], ys[0:w, 0:w_js])
                        else:
                            m = mrg.tile([128, 16], F32)
                            nc.vector.tensor_copy(m[0:w, 0:8], runA[0:w, it, :])
                            nc.vector.max(m[0:w, 8:16], ys[0:w, 0:w_js])
                            nc.vector.max(runA[0:w, it, :], m[0:w, :])

                # ------------- tail: sqrt + softmin weight -------------------
                with tc.tile_pool(name="tails", bufs=4) as tails:
                    for it in range(NIT):
                        w = 128 if it < 12 else LAST_W
                        d3 = tails.tile([128, 3], F32, tag="d3")
                        nc.scalar.activation(
                            d3[0:w, :], runA[0:w, it, 0:3], AF.Sqrt,
                            bias=f_col[0:w, it:it + 1], scale=-1.0,
                        )
                        dd = tails.tile([128, 3], F32, tag="dd")
                        nc.vector.tensor_scalar(
                            dd[0:w, :], d3[0:w, :], d3[0:w, 0:1], None,
                            ALU.subtract,
                        )
                        ee = tails.tile([128, 3], F32, tag="ee")
                        nc.scalar.activation(ee[0:w, :], dd[0:w, :], AF.Exp,
                                             scale=-1.0)
                        ss = tails.tile([128, 1], F32, tag="ss")
                        nc.vector.tensor_reduce(ss[0:w, :], ee[0:w, :], AX.X,
                                                ALU.add)
                        rr = tails.tile([128, 1], F32, tag="rr")
                        nc.vector.reciprocal(rr[0:w, :], ss[0:w, :])
                        nc.vector.tensor_scalar(
                            score_col[0:w, it:it + 1], d3[0:w, 0:1],
                            rr[0:w, 0:1], None, ALU.mult,
                        )
            nc.sync.dma_start(score_d[:], score_col[:])

    nc.compile()
    return nc


def _get_program():
    if "nc" not in _cache:
        _cache["nc"] = _build_program()
    return _cache["nc"]


def kernel(p, W, b, C):
    from concourse.bass_utils import run_bass_kernel_spmd

    nc = _get_program()

    p = np.ascontiguousarray(np.asarray(p, dtype=np.float32))
    W = np.asarray(W, dtype=np.float32)
    b = np.ascontiguousarray(np.asarray(b, dtype=np.float32))
    C = np.ascontiguousarray(np.asarray(C, dtype=np.float32))

    wt = np.ascontiguousarray(W.T)                                # [c, d]
    cn = np.sum(C.astype(np.float64) * C, axis=0).astype(np.float32)
    cblock = np.broadcast_to((-cn / 128.0)[None, :], (128, P))
    cb = np.ascontiguousarray(
        np.concatenate([2.0 * C, cblock], axis=0)                 # [1920, P]
    )
    onec = np.ones((128, 1), dtype=np.float32)
    oner = np.ones((1, 128), dtype=np.float32)
    ones2 = np.ones((128, 128), dtype=np.float32)

    p_flat = p.reshape(B, DIM, HW)
    in_maps = []
    for core in range(NCORES):
        bidx, half = divmod(core, 2)
        pT = np.ascontiguousarray(p_flat[bidx, :, half * HALF:(half + 1) * HALF])
        in_maps.append({
            "pT": pT, "wt": wt, "cb": cb, "bias": b,
            "onec": onec, "oner": oner, "ones2": ones2,
        })

    _cache["last_in_maps"] = in_maps
    res = run_bass_kernel_spmd(nc, in_maps, list(range(NCORES)))
    _cache["last_result"] = res

    return assemble_output(per_core=[res.results[c]["score"] for c in range(NCORES)])


def assemble_output(per_core=None, res_concat=None):
    if per_core is None:
        sc_all = res_concat["score"]                              # [8*128, 13]
        per_core = [sc_all[c * 128:(c + 1) * 128] for c in range(NCORES)]
    out = np.empty((B, 1, H, W_), dtype=np.float32)
    for core in range(NCORES):
        bidx, half = divmod(core, 2)
        sc = per_core[core]                                       # [128, 13]
        flat = np.empty(HALF, dtype=np.float32)
        flat[:12 * 128] = sc[:, :12].T.reshape(-1)
        flat[12 * 128:] = sc[:LAST_W, 12]
        out.reshape(B, 1, HW)[bidx, 0, half * HALF:(half + 1) * HALF] = flat
    return out



# revision 4
# speedup vs baseline: 1.9011x; 1.9011x over previous
"""Trainium2 Bass kernel for nn_DSVDD (retrieval_knn).

Math (per batch b):
  phi = W @ p_b + bias            [DIM, HW]    (1x1 conv)
  sqdist[i,j] = ||phi_i||^2 + ||C_j||^2 - 2 phi_i . C_j
  top-3 smallest distances d0<=d1<=d2  ->  w0 = 1/(1+exp(d0-d1)+exp(d0-d2))
  score[i] = w0 * d0

Device strategy (8 cores, data-parallel over (batch, HW-half)):
  All heavy matmuls run in fp8e4 with perf_mode=DoubleRow (2 contraction
  rows/cycle, ~1.7x over fp32r at FD>=256).  Host prescales W by 64 and C
  by 128 so fp8 operands sit in the normal range (sigma ~1-3); the scale
  is divided back out on the ACT path (phi = psum/64) and in the final
  sqrt (d = sqrt(f - psum/64)).  Y[i,j] = 64*(2 phi_i . C_j - c_j); the
  -c_j correction is materialized once per j-slice via a ones-matmul on
  the replicated -c/2 block and applied by a DVE add.  top-3 smallest
  sqdist == top-3 largest Y (f_i = ||phi_i||^2 common per row).  DVE max8
  finds the top-8 largest Y per row; streamed merge over j-slices.  f_i
  via fp32 ones-matmuls over Square(psum/64) so f keeps full accuracy.
  Tail (sqrt, softmin) on ACT/DVE.  rel-err budget 2e-2 >> fp8 noise.
"""
import sys

sys.path.insert(0, "/opt/trn_rl_repo")

import numpy as np
import ml_dtypes

B, DIM, H, W_ = 4, 1792, 56, 56
HW = H * W_            # 3136
P = 3136               # prototypes
NCORES = 8
HALF = HW // 2         # 1568 positions per core
KC = DIM // 128        # 14 contraction chunks
NPAIR = KC // 2        # 7 DoubleRow pairs
KCC = KC + 1           # 15 chunks in cb (incl. replicated -c/2 block)
IB = 392               # conv i-block (moving cols)
IBPAD = 400            # p tile inner pad (DoubleRow needs 16B-mult stride)
NIB = HALF // IB       # 4
JSLICES = [256, 480, 480, 480, 480, 480, 480]   # G-phase j-slices (sum 3136)
NIT = 13               # i-tiles: 12 full + 1 ragged(32)
LAST_W = HALF - 12 * 128   # 32
PHIPAD = NIT * 128     # 1664 (phi padded so the ragged i-tile is uniform)
WSCALE = 64.0          # host prescale on W (and 2C -> 128C)

_cache = {}


def _build_program():
    import concourse.tile as tile
    from concourse import bacc, mybir

    F32 = mybir.dt.float32
    F32R = mybir.dt.float32r
    F8 = mybir.dt.float8e4
    AF = mybir.ActivationFunctionType
    ALU = mybir.AluOpType
    AX = mybir.AxisListType
    DR = mybir.MatmulPerfMode.DoubleRow

    nc = bacc.Bacc("TRN2", target_bir_lowering=False, debug=False)

    pT_d = nc.dram_tensor("pT", [DIM, HALF], F8, kind="ExternalInput")
    wt_d = nc.dram_tensor("wt", [KC, 128, KC, 128], F8, kind="ExternalInput")
    cb_d = nc.dram_tensor("cb", [KCC * 128, P], F8, kind="ExternalInput")
    bias_d = nc.dram_tensor("bias", [DIM], F32, kind="ExternalInput")
    onec_d = nc.dram_tensor("onec", [128, 1], F32R, kind="ExternalInput")
    oner_d = nc.dram_tensor("oner", [1, 128], F32R, kind="ExternalInput")
    ones2_d = nc.dram_tensor("ones2", [128, 128], F8, kind="ExternalInput")
    score_d = nc.dram_tensor("score", [128, NIT], F32, kind="ExternalOutput")

    with tile.TileContext(nc) as tc:
        with (
            tc.tile_pool(name="persist", bufs=1) as persist,
            tc.tile_pool(name="cbp0", bufs=1) as cbp0,
        ):
            phi = persist.tile([128, KC, PHIPAD], F8)
            bias_col = persist.tile([128, KC], F32)
            onec = persist.tile([128, 1], F32R)
            oner = persist.tile([1, 128], F32R)
            ones2 = persist.tile([128, 128], F8)
            f_row = persist.tile([1, HALF], F32)
            f_col = persist.tile([128, NIT], F32)
            runA = persist.tile([128, NIT, 8], F32)
            score_col = persist.tile([128, NIT], F32)

            # ------------- conv phase: phi = W @ p + b, f = ||phi||^2 -------
            with (
                tc.tile_pool(name="pp", bufs=6) as pp,
                tc.tile_pool(name="wtp", bufs=3) as wtp,
                tc.tile_pool(name="sqp", bufs=4) as sqp,
                tc.tile_pool(name="cps", bufs=4, space="PSUM") as cps,
                tc.tile_pool(name="fps", bufs=1, space="PSUM") as fps,
            ):
                f_ps = [fps.tile([1, IB], F32, name=f"fp{ib}", tag=f"f{ib}")
                        for ib in range(NIB)]

                def load_wt(dcg):
                    t = wtp.tile([128, KC, 128], F8, name="wt_t")
                    nc.sync.dma_start(t[:], wt_d[dcg])
                    return t

                def load_ptile(ib):
                    # two DMAs into one tile: cc 0..7 (DR pairs 0-3) lands
                    # first for early start, then cc 8..13 (pairs 4-6)
                    pt = pp.tile([128, KC, IBPAD], F8, name=f"pq{ib}",
                                 tag="pq")
                    for cc0, cc1 in ((0, 8), (8, KC)):
                        nc.sync.dma_start(
                            pt[:, cc0:cc1, 0:IB],
                            pT_d[cc0 * 128:cc1 * 128,
                                 ib * IB:(ib + 1) * IB].rearrange(
                                "(cc p) i -> p cc i", p=128),
                        )
                    return pt

                # startup-critical loads first: wt chunk 0, then p halves
                dcg_seq = list(range(KC)) + list(reversed(range(KC)))  # snake
                wt_tiles = {0: load_wt(dcg_seq[0])}
                wt_issued = 1

                def wt_prefetch(upto):
                    nonlocal wt_issued
                    while wt_issued < min(upto, 2 * KC):
                        if dcg_seq[wt_issued] == dcg_seq[wt_issued - 1]:
                            # snake turn: same chunk again, reuse the tile
                            wt_tiles[wt_issued] = wt_tiles[wt_issued - 1]
                        else:
                            wt_tiles[wt_issued] = load_wt(dcg_seq[wt_issued])
                        wt_issued += 1

                # PE warmup: dummy matmuls keep HAM's activity monitor hot
                # while the first real DMAs land, so conv starts at 2.4 GHz.
                warm = pp.tile([128, 512], F32R, name="warm", tag="warm", bufs=1)
                nc.vector.memset(warm[:].bitcast(F32), 1.0)
                wps = cps.tile([128, 512], F32, name="wps", tag="acc")
                for _ in range(24):
                    nc.tensor.matmul(wps[:], warm[:, 0:128], warm[:],
                                     start=True, stop=True)
                # zero the phi pad columns so the uniform last i-tile reads 0s
                nc.vector.memset(phi[:, :, HALF:PHIPAD].bitcast(F32), 0.0)

                cb0_t = None
                small_dmas_done = False
                pending_f = []
                for sub in range(2):
                    p_t = {}
                    for ib in (2 * sub, 2 * sub + 1):
                        p_t[ib] = load_ptile(ib)
                    if not small_dmas_done:
                        small_dmas_done = True
                        nc.sync.dma_start(
                            bias_col[:],
                            bias_d.rearrange("(g p) -> p g", p=128))
                        nc.sync.dma_start(onec[:], onec_d[:])
                        nc.sync.dma_start(oner[:], oner_d[:])
                        nc.sync.dma_start(ones2[:], ones2_d[:])
                    for dcg_i in range(KC):
                        pos = sub * KC + dcg_i
                        dcg = dcg_seq[pos]
                        wt_t = wt_tiles.pop(pos)
                        wt_prefetch(pos + 3)
                        for k, ib in enumerate((2 * sub, 2 * sub + 1)):
                            if k == 1 and pending_f:
                                # deferred f matmuls: deps long satisfied
                                for args, kw in pending_f:
                                    nc.tensor.matmul(*args, **kw)
                                pending_f = []
                            acc = cps.tile([128, IB], F32)
                            for pc in range(NPAIR):
                                nc.tensor.matmul(
                                    acc[:],
                                    wt_t[:, 2 * pc:2 * pc + 2, :],
                                    p_t[ib][:, 2 * pc:2 * pc + 2, 0:IB],
                                    start=(pc == 0),
                                    stop=(pc == NPAIR - 1),
                                    perf_mode=DR,
                                )
                            isl = slice(ib * IB, (ib + 1) * IB)
                            # phi = psum/64 + bias (rounded to fp8)
                            nc.scalar.activation(
                                phi[:, dcg, isl], acc[:], AF.Identity,
                                bias=bias_col[:, dcg:dcg + 1],
                                scale=1.0 / WSCALE,
                            )
                            # phi2 = (psum/64 + bias)^2
                            sq = sqp.tile([128, IB], F32R)
                            nc.scalar.activation(
                                sq[:], acc[:], AF.Square,
                                bias=bias_col[:, dcg:dcg + 1],
                                scale=1.0 / WSCALE,
                            )
                            pending_f.append((
                                (f_ps[ib][:], onec[:], sq[:]),
                                dict(start=(dcg_i == 0), stop=(dcg_i == KC - 1)),
                            ))
                    if sub == 0:
                        # prefetch first G slice mid-conv
                        j0 = JSLICES[0]
                        cb0_t = cbp0.tile([128, KCC, j0], F8)
                        nc.sync.dma_start(
                            cb0_t[:],
                            cb_d[:, 0:j0].rearrange("(cc p) j -> p cc j",
                                                    p=128),
                        )
                for args, kw in pending_f:
                    nc.tensor.matmul(*args, **kw)
                pending_f = []
                for ib in range(NIB):
                    nc.vector.tensor_copy(
                        f_row[:, ib * IB:(ib + 1) * IB], f_ps[ib][:]
                    )

            # ------------- f relayout: [1, 1568] -> [128, 13] ---------------
            with tc.tile_pool(name="ftp", bufs=2, space="PSUM") as ftp:
                ft = ftp.tile([128, NIT], F32)
                for it in range(NIT):
                    w = 128 if it < 12 else LAST_W
                    nc.tensor.transpose(
                        ft[0:w, it:it + 1],
                        f_row[:, it * 128:it * 128 + w],
                        oner[0:1, 0:1].bitcast(F32),
                    )
                nc.scalar.activation(f_col[:], ft[:], AF.Copy)

            # ------------- G phase: Y = 64(2 phi.C - c), streamed top-8 -----
            with (
                tc.tile_pool(name="cbp", bufs=2) as cbp,
                tc.tile_pool(name="cbcp", bufs=2) as cbcp,
                tc.tile_pool(name="ysb", bufs=4) as ysb,
                tc.tile_pool(name="mrg", bufs=4) as mrg,
                tc.tile_pool(name="yps", bufs=8, space="PSUM") as yps,
            ):
                joff = [0]
                for js in range(1, len(JSLICES)):
                    joff.append(joff[-1] + JSLICES[js - 1])

                for js in range(len(JSLICES)):
                    w_js = JSLICES[js]
                    jsl = slice(joff[js], joff[js] + w_js)
                    if js == 0:
                        cb_t = cb0_t
                    else:
                        cb_t = cbp.tile([128, KCC, w_js], F8, name="cb_t",
                                        tag="cb")
                        nc.sync.dma_start(
                            cb_t[:],
                            cb_d[:, jsl].rearrange("(cc p) j -> p cc j",
                                                   p=128),
                        )
                    # materialize -64c for this slice: ones2 @ (-c/2 block)
                    cps_t = yps.tile([128, 512], F32, name="y", tag="y")
                    nc.tensor.matmul(cps_t[:, 0:w_js], ones2[:],
                                     cb_t[:, KC, :], start=True, stop=True)
                    cbc_t = cbcp.tile([128, 512], F32, name="cbc_t")
                    nc.scalar.activation(cbc_t[:, 0:w_js], cps_t[:, 0:w_js],
                                         AF.Copy)
                    for it in range(NIT):
                        i0 = it * 128
                        y = yps.tile([128, 512], F32, name="y", tag="y")
                        for pc in range(NPAIR):
                            nc.tensor.matmul(
                                y[:, 0:w_js],
                                phi[:, 2 * pc:2 * pc + 2, i0:i0 + 128],
                                cb_t[:, 2 * pc:2 * pc + 2, :],
                                start=(pc == 0),
                                stop=(pc == NPAIR - 1),
                                perf_mode=DR,
                            )
                        ys = ysb.tile([128, 512], F32, name="ys", tag="ys")
                        nc.vector.tensor_tensor(
                            ys[:, 0:w_js], y[:, 0:w_js],
                            cbc_t[:, 0:w_js], ALU.add,
                        )
                        if js == 0:
                            nc.vector.max(runA[:, it, :], ys[:, 0:w_js])
                        else:
                            m = mrg.tile([128, 16], F32)
                            nc.vector.tensor_copy(m[:, 0:8], runA[:, it, :])
                            nc.vector.max(m[:, 8:16], ys[:, 0:w_js])
                            nc.vector.max(runA[:, it, :], m[:, :])

                # ------------- tail: sqrt + softmin weight -------------------
                with tc.tile_pool(name="tails", bufs=4) as tails:
                    for it in range(NIT):
                        d3 = tails.tile([128, 3], F32, tag="d3")
                        nc.scalar.activation(
                            d3[:, :], runA[:, it, 0:3], AF.Sqrt,
                            bias=f_col[:, it:it + 1], scale=-1.0 / WSCALE,
                        )
                        dd = tails.tile([128, 3], F32, tag="dd")
                        nc.vector.tensor_scalar(
                            dd[:, :], d3[:, :], d3[:, 0:1], None,
                            ALU.subtract,
                        )
                        ee = tails.tile([128, 3], F32, tag="ee")
                        nc.scalar.activation(ee[:, :], dd[:, :], AF.Exp,
                                             scale=-1.0)
                        ss = tails.tile([128, 1], F32, tag="ss")
                        nc.vector.tensor_reduce(ss[:, :], ee[:, :], AX.X,
                                                ALU.add)
                        rr = tails.tile([128, 1], F32, tag="rr")
                        nc.vector.reciprocal(rr[:, :], ss[:, :])
                        nc.vector.tensor_scalar(
                            score_col[:, it:it + 1], d3[:, 0:1],
                            rr[:, 0:1], None, ALU.mult,
                        )
            nc.sync.dma_start(score_d[:], score_col[:])

    nc.compile()
    return nc


def _get_program():
    if "nc" not in _cache:
        _cache["nc"] = _build_program()
    return _cache["nc"]


def kernel(p, W, b, C):
    from concourse.bass_utils import run_bass_kernel_spmd

    nc = _get_program()

    F8NP = ml_dtypes.float8_e4m3

    p = np.ascontiguousarray(np.asarray(p, dtype=np.float32))
    W = np.asarray(W, dtype=np.float32)
    b = np.ascontiguousarray(np.asarray(b, dtype=np.float32))
    C = np.ascontiguousarray(np.asarray(C, dtype=np.float32))

    # wt[dcg, pin, cc, d] = 64*W[dcg*128+d, cc*128+pin]
    wt = np.ascontiguousarray(
        (WSCALE * W).reshape(KC, 128, KC, 128).transpose(0, 3, 2, 1)
    ).astype(F8NP)
    cn = np.sum(C.astype(np.float64) * C, axis=0).astype(np.float32)
    cb = np.empty((KCC * 128, P), dtype=F8NP)
    cb[:DIM] = (2.0 * WSCALE * C).astype(F8NP)
    cb[DIM:] = np.broadcast_to((-cn / 2.0)[None, :], (128, P)).astype(F8NP)
    onec = np.ones((128, 1), dtype=np.float32)
    oner = np.ones((1, 128), dtype=np.float32)
    ones2 = np.ones((128, 128), dtype=F8NP)

    p_flat = p.reshape(B, DIM, HW)
    in_maps = []
    for core in range(NCORES):
        bidx, half = divmod(core, 2)
        pT = np.ascontiguousarray(
            p_flat[bidx, :, half * HALF:(half + 1) * HALF]).astype(F8NP)
        in_maps.append({
            "pT": pT, "wt": wt, "cb": cb, "bias": b,
            "onec": onec, "oner": oner, "ones2": ones2,
        })

    _cache["last_in_maps"] = in_maps
    res = run_bass_kernel_spmd(nc, in_maps, list(range(NCORES)))
    _cache["last_result"] = res

    return assemble_output(per_core=[res.results[c]["score"] for c in range(NCORES)])


def assemble_output(per_core=None, res_concat=None):
    if per_core is None:
        sc_all = res_concat["score"]                              # [8*128, 13]
        per_core = [sc_all[c * 128:(c + 1) * 128] for c in range(NCORES)]
    out = np.empty((B, 1, H, W_), dtype=np.float32)
    for core in range(NCORES):
        bidx, half = divmod(core, 2)
        sc = per_core[core]                                       # [128, 13]
        flat = np.empty(HALF, dtype=np.float32)
        flat[:12 * 128] = sc[:, :12].T.reshape(-1)
        flat[12 * 128:] = sc[:LAST_W, 12]
        out.reshape(B, 1, HW)[bidx, 0, half * HALF:(half + 1) * HALF] = flat
    return out


# revision 15
# speedup vs baseline: 2.0885x; 1.0986x over previous
"""Trainium2 Bass kernel for nn_DSVDD (retrieval_knn).

Math (per batch b):
  phi = W @ p_b + bias            [DIM, HW]    (1x1 conv)
  sqdist[i,j] = ||phi_i||^2 + ||C_j||^2 - 2 phi_i . C_j
  top-3 smallest distances d0<=d1<=d2  ->  w0 = 1/(1+exp(d0-d1)+exp(d0-d2))
  score[i] = w0 * d0

Device strategy (8 cores, data-parallel over (batch, HW-half)):
  All heavy matmuls run in fp8e4 with perf_mode=DoubleRow (2 contraction
  rows/cycle, ~1.7x over fp32r at FD>=256).  Host prescales W by 64 and C
  by 128 so fp8 operands sit in the normal range (sigma ~1-3); the scale
  is divided back out on the ACT path (phi = psum/64) and in the final
  sqrt (d = sqrt(f - psum/64)).  Y[i,j] = 64*(2 phi_i . C_j - c_j); the
  -c_j correction is materialized once per j-slice via a ones-matmul on
  the replicated -c/2 block and applied by a DVE add.  top-3 smallest
  sqdist == top-3 largest Y (f_i = ||phi_i||^2 common per row).  DVE max8
  finds the top-8 largest Y per row; streamed merge over j-slices.  f_i
  via fp32 ones-matmuls over Square(psum/64) so f keeps full accuracy.
  Tail (sqrt, softmin) on ACT/DVE.  rel-err budget 2e-2 >> fp8 noise.
"""
import sys

sys.path.insert(0, "/opt/trn_rl_repo")

import numpy as np
import ml_dtypes

B, DIM, H, W_ = 4, 1792, 56, 56
HW = H * W_            # 3136
P = 3136               # prototypes
NCORES = 8
HALF = HW // 2         # 1568 positions per core
KC = DIM // 128        # 14 contraction chunks
NPAIR = KC // 2        # 7 DoubleRow pairs
KCC = KC + 1           # 15 chunks in cb (incl. replicated -c/2 block)
IB = 392               # conv i-block (moving cols)
IBPAD = 400            # p tile inner pad (DoubleRow needs 16B-mult stride)
NIB = HALF // IB       # 4
JSLICES = [256, 480, 480, 480, 480, 480, 480]   # G-phase j-slices (sum 3136)
NIT = 13               # i-tiles: 12 full + 1 ragged(32)
LAST_W = HALF - 12 * 128   # 32
PHIPAD = NIT * 128     # 1664 (phi padded so the ragged i-tile is uniform)
WSCALE = 64.0          # host prescale on W (and 2C -> 128C)

_cache = {}


def _build_program():
    import concourse.tile as tile
    from concourse import bacc, mybir

    F32 = mybir.dt.float32
    F32R = mybir.dt.float32r
    F8 = mybir.dt.float8e4
    AF = mybir.ActivationFunctionType
    ALU = mybir.AluOpType
    AX = mybir.AxisListType
    DR = mybir.MatmulPerfMode.DoubleRow

    nc = bacc.Bacc("TRN2", target_bir_lowering=False, debug=False)

    pT_d = nc.dram_tensor("pT", [DIM, HALF], F8, kind="ExternalInput")
    wt_d = nc.dram_tensor("wt", [KC, 128, KC, 128], F8, kind="ExternalInput")
    cb_d = nc.dram_tensor("cb", [KCC * 128, P], F8, kind="ExternalInput")
    oner_d = nc.dram_tensor("oner", [1, 128], F32R, kind="ExternalInput")
    ones2_d = nc.dram_tensor("ones2", [128, 2, 64], F8, kind="ExternalInput")
    score_d = nc.dram_tensor("score", [128, NIT], F32, kind="ExternalOutput")

    with tile.TileContext(nc) as tc:
        with (
            tc.tile_pool(name="persist", bufs=1) as persist,
            tc.tile_pool(name="cbp0", bufs=1) as cbp0,
        ):
            phi = persist.tile([128, KC, PHIPAD], F8)
            oner = persist.tile([1, 128], F32R)
            ones2 = persist.tile([128, 2, 64], F8)
            f_row = persist.tile([1, HALF], F32)
            f_col = persist.tile([128, NIT], F32)
            runA = persist.tile([128, NIT, 8], F32)
            score_col = persist.tile([128, NIT], F32)

            # ------------- conv phase: phi = W @ p + b, f = ||phi||^2 -------
            with (
                tc.tile_pool(name="pp", bufs=6) as pp,
                tc.tile_pool(name="wtp", bufs=3) as wtp,
                tc.tile_pool(name="sqp", bufs=4) as sqp,
                tc.tile_pool(name="cps", bufs=4, space="PSUM") as cps,
                tc.tile_pool(name="fps", bufs=1, space="PSUM") as fps,
            ):
                f_ps = [fps.tile([1, IB], F32, name=f"fp{ib}", tag=f"f{ib}")
                        for ib in range(NIB)]

                def load_wt(dcg):
                    t = wtp.tile([128, KC, 128], F8, name="wt_t")
                    nc.sync.dma_start(t[:], wt_d[dcg])
                    return t

                def load_ptile(ib):
                    # two DMAs into one tile: cc 0..7 (DR pairs 0-3) lands
                    # first for early start, then cc 8..13 (pairs 4-6)
                    pt = pp.tile([128, KC, IBPAD], F8, name=f"pq{ib}",
                                 tag="pq")
                    for cc0, cc1 in ((0, 8), (8, KC)):
                        nc.sync.dma_start(
                            pt[:, cc0:cc1, 0:IB],
                            pT_d[cc0 * 128:cc1 * 128,
                                 ib * IB:(ib + 1) * IB].rearrange(
                                "(cc p) i -> p cc i", p=128),
                        )
                    return pt

                # startup-critical loads first: wt chunk 0, then p halves
                dcg_seq = list(range(KC)) + list(reversed(range(KC)))  # snake
                wt_tiles = {0: load_wt(dcg_seq[0])}
                wt_issued = 1

                def wt_prefetch(upto):
                    nonlocal wt_issued
                    while wt_issued < min(upto, 2 * KC):
                        if dcg_seq[wt_issued] == dcg_seq[wt_issued - 1]:
                            # snake turn: same chunk again, reuse the tile
                            wt_tiles[wt_issued] = wt_tiles[wt_issued - 1]
                        else:
                            wt_tiles[wt_issued] = load_wt(dcg_seq[wt_issued])
                        wt_issued += 1

                # PE warmup: dummy matmuls keep HAM's activity monitor hot
                # while the first real DMAs land, so conv starts at full clock.
                warm = pp.tile([128, 512], F32R, name="warm", tag="warm", bufs=1)
                nc.vector.memset(warm[:].bitcast(F32), 1.0)
                wps = cps.tile([128, 512], F32, name="wps", tag="acc")
                for _ in range(12):
                    nc.tensor.matmul(wps[:], warm[:, 0:128], warm[:],
                                     start=True, stop=True)
                # zero the phi pad columns so the uniform last i-tile reads 0s
                nc.vector.memset(phi[:, :, HALF:PHIPAD].bitcast(F32), 0.0)

                cb0_t = None
                small_dmas_done = False
                pending_f = []
                _sq_cur = {}
                for sub in range(2):
                    p_t = {}
                    # interleave wt prefetch with p loads: each DMA issue
                    # costs ~700ns on Sync, and wt1 is needed ~2.3us in
                    p_t[2 * sub] = load_ptile(2 * sub)
                    if sub == 0:
                        wt_prefetch(2)
                    p_t[2 * sub + 1] = load_ptile(2 * sub + 1)
                    if not small_dmas_done:
                        small_dmas_done = True
                        wt_prefetch(3)
                        nc.sync.dma_start(oner[:], oner_d[:])
                        nc.sync.dma_start(ones2[:], ones2_d[:])
                    for dcg_i in range(KC):
                        pos = sub * KC + dcg_i
                        dcg = dcg_seq[pos]
                        wt_t = wt_tiles.pop(pos)
                        wt_prefetch(pos + 3)
                        for k, ib in enumerate((2 * sub, 2 * sub + 1)):
                            if k == 1 and pending_f:
                                # deferred f matmuls: deps long satisfied
                                for args, kw in pending_f:
                                    nc.tensor.matmul(*args, **kw)
                                pending_f = []
                            acc = cps.tile([128, IB], F32)
                            for pc in range(NPAIR):
                                nc.tensor.matmul(
                                    acc[:],
                                    wt_t[:, 2 * pc:2 * pc + 2, :],
                                    p_t[ib][:, 2 * pc:2 * pc + 2, 0:IB],
                                    start=(pc == 0),
                                    stop=(pc == NPAIR - 1),
                                    perf_mode=DR,
                                )
                            isl = slice(ib * IB, (ib + 1) * IB)
                            # phi = psum/64 (bias folded into C on host)
                            nc.scalar.activation(
                                phi[:, dcg, isl], acc[:], AF.Identity,
                                scale=1.0 / WSCALE,
                            )
                            # phi2 = (psum/64)^2 in fp8 (values 0..~30),
                            # paired along dcg for a DoubleRow f-matmul
                            if dcg_i % 2 == 0:
                                sq = sqp.tile([128, 2, IBPAD], F8)
                                _sq_cur[ib] = sq
                            else:
                                sq = _sq_cur[ib]
                            nc.scalar.activation(
                                sq[:, dcg_i % 2, 0:IB], acc[:], AF.Square,
                                scale=1.0 / WSCALE,
                            )
                            if dcg_i % 2 == 1:
                                pending_f.append((
                                    (f_ps[ib][:], ones2[:, 0:2, 0:1],
                                     sq[:, 0:2, 0:IB]),
                                    dict(start=(dcg_i == 1),
                                         stop=(dcg_i == KC - 1),
                                         perf_mode=DR),
                                ))
                    if sub == 0:
                        # prefetch first G slice mid-conv
                        j0 = JSLICES[0]
                        cb0_t = cbp0.tile([128, KCC, j0], F8)
                        nc.sync.dma_start(
                            cb0_t[:],
                            cb_d[:, 0:j0].rearrange("(cc p) j -> p cc j",
                                                    p=128),
                        )
                for args, kw in pending_f:
                    nc.tensor.matmul(*args, **kw)
                pending_f = []
                for ib in range(NIB):
                    nc.vector.tensor_copy(
                        f_row[:, ib * IB:(ib + 1) * IB], f_ps[ib][:]
                    )

            # ------------- f relayout: [1, 1568] -> [128, 13] ---------------
            with tc.tile_pool(name="ftp", bufs=2, space="PSUM") as ftp:
                ft = ftp.tile([128, NIT], F32)
                for it in range(NIT):
                    w = 128 if it < 12 else LAST_W
                    nc.tensor.transpose(
                        ft[0:w, it:it + 1],
                        f_row[:, it * 128:it * 128 + w],
                        oner[0:1, 0:1].bitcast(F32),
                    )
                # f_col holds 64*f so the tail subtract needs no rescale
                nc.scalar.activation(f_col[:], ft[:], AF.Copy,
                                     scale=WSCALE)

            # ------------- G phase: Y = 64(2 phi.C - c), streamed top-8 -----
            with (
                tc.tile_pool(name="cbp", bufs=2) as cbp,
                tc.tile_pool(name="cbcp", bufs=2) as cbcp,
                tc.tile_pool(name="ysb", bufs=4) as ysb,
                tc.tile_pool(name="mrg", bufs=4) as mrg,
                tc.tile_pool(name="yps", bufs=7, space="PSUM") as yps,
                tc.tile_pool(name="ccps", bufs=1, space="PSUM") as ccps,
            ):
                joff = [0]
                for js in range(1, len(JSLICES)):
                    joff.append(joff[-1] + JSLICES[js - 1])

                for js in range(len(JSLICES)):
                    w_js = JSLICES[js]
                    jsl = slice(joff[js], joff[js] + w_js)
                    if js == 0:
                        cb_t = cb0_t
                    else:
                        cb_t = cbp.tile([128, KCC, w_js], F8, name="cb_t",
                                        tag="cb")
                        nc.sync.dma_start(
                            cb_t[:],
                            cb_d[:, jsl].rearrange("(cc p) j -> p cc j",
                                                   p=128),
                        )
                    # materialize -64c for this slice: ones2 @ (-c/2 block)
                    cps_t = ccps.tile([128, 512], F32, name="cps")
                    nc.tensor.matmul(cps_t[:, 0:w_js], ones2[:],
                                     cb_t[:, KC, :], start=True, stop=True)
                    cbc_t = cbcp.tile([128, 512], F32, name="cbc_t")
                    nc.scalar.activation(cbc_t[:, 0:w_js], cps_t[:, 0:w_js],
                                         AF.Copy)
                    for it in range(NIT):
                        i0 = it * 128
                        y = yps.tile([128, 512], F32, name="y", tag="y")
                        for pc in range(NPAIR):
                            nc.tensor.matmul(
                                y[:, 0:w_js],
                                phi[:, 2 * pc:2 * pc + 2, i0:i0 + 128],
                                cb_t[:, 2 * pc:2 * pc + 2, :],
                                start=(pc == 0),
                                stop=(pc == NPAIR - 1),
                                perf_mode=DR,
                            )
                        ys = ysb.tile([128, 512], F32, name="ys", tag="ys")
                        nc.vector.tensor_tensor(
                            ys[:, 0:w_js], y[:, 0:w_js],
                            cbc_t[:, 0:w_js], ALU.add,
                        )
                        if js == 0:
                            nc.vector.max(runA[:, it, :], ys[:, 0:w_js])
                        else:
                            m = mrg.tile([128, 16], F32)
                            nc.vector.tensor_copy(m[:, 0:8], runA[:, it, :])
                            nc.vector.max(m[:, 8:16], ys[:, 0:w_js])
                            nc.vector.max(runA[:, it, :], m[:, :])

                # ---- tail: sqrt + softmin weight, batched over all i-tiles --
                with tc.tile_pool(name="tails", bufs=2) as tails:
                    t64 = tails.tile([128, NIT, 3], F32, tag="t64")
                    nc.vector.tensor_tensor(
                        t64[:], f_col[:, :, None].broadcast_to([128, NIT, 3]),
                        runA[:, :, 0:3], ALU.subtract,
                    )
                    d3a = tails.tile([128, NIT, 3], F32, tag="d3a")
                    nc.scalar.activation(d3a[:], t64[:], AF.Sqrt,
                                         scale=1.0 / WSCALE)
                    dd = tails.tile([128, NIT, 3], F32, tag="dd")
                    nc.vector.tensor_tensor(
                        dd[:], d3a[:],
                        d3a[:, :, 0:1].broadcast_to([128, NIT, 3]),
                        ALU.subtract,
                    )
                    ee = tails.tile([128, NIT, 3], F32, tag="ee")
                    nc.scalar.activation(ee[:], dd[:], AF.Exp, scale=-1.0)
                    ss = tails.tile([128, NIT], F32, tag="ss")
                    nc.vector.tensor_reduce(ss[:], ee[:], AX.X, ALU.add)
                    rr = tails.tile([128, NIT], F32, tag="rr")
                    nc.vector.reciprocal(rr[:], ss[:])
                    nc.vector.tensor_tensor(
                        score_col[:], d3a[:, :, 0], rr[:], ALU.mult,
                    )
            nc.sync.dma_start(score_d[:], score_col[:])

    nc.compile()
    return nc


def _get_program():
    if "nc" not in _cache:
        _cache["nc"] = _build_program()
    return _cache["nc"]


def kernel(p, W, b, C):
    from concourse.bass_utils import run_bass_kernel_spmd

    nc = _get_program()

    F8NP = ml_dtypes.float8_e4m3

    p = np.ascontiguousarray(np.asarray(p, dtype=np.float32))
    W = np.asarray(W, dtype=np.float32)
    b = np.ascontiguousarray(np.asarray(b, dtype=np.float32))
    C = np.ascontiguousarray(np.asarray(C, dtype=np.float32))

    # fold the conv bias into the prototypes: ||(Wp+b) - C_j|| =
    # ||Wp - (C_j - b)||, so the device kernel needs no bias path
    Cs = C - b[:, None]

    # wt[dcg, pin, cc, d] = 64*W[dcg*128+d, cc*128+pin]
    wt = np.ascontiguousarray(
        (WSCALE * W).reshape(KC, 128, KC, 128).transpose(0, 3, 2, 1)
    ).astype(F8NP)
    cn = np.sum(Cs.astype(np.float64) * Cs, axis=0).astype(np.float32)
    cb = np.empty((KCC * 128, P), dtype=F8NP)
    cb[:DIM] = (2.0 * WSCALE * Cs).astype(F8NP)
    cb[DIM:] = np.broadcast_to((-cn / 2.0)[None, :], (128, P)).astype(F8NP)
    oner = np.ones((1, 128), dtype=np.float32)
    ones2 = np.ones((128, 2, 64), dtype=F8NP)

    p_flat = p.reshape(B, DIM, HW)
    in_maps = []
    for core in range(NCORES):
        bidx, half = divmod(core, 2)
        pT = np.ascontiguousarray(
            p_flat[bidx, :, half * HALF:(half + 1) * HALF]).astype(F8NP)
        in_maps.append({
            "pT": pT, "wt": wt, "cb": cb,
            "oner": oner, "ones2": ones2,
        })

    _cache["last_in_maps"] = in_maps
    res = run_bass_kernel_spmd(nc, in_maps, list(range(NCORES)))
    _cache["last_result"] = res

    return assemble_output(per_core=[res.results[c]["score"] for c in range(NCORES)])


def assemble_output(per_core=None, res_concat=None):
    if per_core is None:
        sc_all = res_concat["score"]                              # [8*128, 13]
        per_core = [sc_all[c * 128:(c + 1) * 128] for c in range(NCORES)]
    out = np.empty((B, 1, H, W_), dtype=np.float32)
    for core in range(NCORES):
        bidx, half = divmod(core, 2)
        sc = per_core[core]                                       # [128, 13]
        flat = np.empty(HALF, dtype=np.float32)
        flat[:12 * 128] = sc[:, :12].T.reshape(-1)
        flat[12 * 128:] = sc[:LAST_W, 12]
        out.reshape(B, 1, HW)[bidx, 0, half * HALF:(half + 1) * HALF] = flat
    return out
